# revision 26
# baseline (speedup 1.0000x reference)
"""CasRel-style kernel for Trainium2 (Bass/Tile), 8-core data-parallel.

Model (per batch b):
  hn_b      = masked LSTM over embed[b, head_b : head_b+16, :]  (16 steps, H=768)
  w_b       = hn_b @ cln_Ww.T + cln_weight ; bvec_b = hn_b @ cln_Wb.T + cln_bias
  ne[b,s]   = w_b * (x - mu)/std + bvec_b          (layernorm over H)
  heads     = sigmoid(ne @ Wh.T + bh) ; tails = sigmoid(ne @ Wt.T + bt)

Kernel strategy (per core, 8 local batches):
  - LN + classifier folded into one PSUM accumulation on raw x:
      logits = a_s * (Wc_b @ x_s - mu_s * vh_b + std_s * c_b),  a_s = 1/std_s
    where Wc_b = [Wh|Wt] * w_b (row-scaled in T layout), vh_b = Wc_b @ 1,
    c_b = [Wh|Wt] @ bvec_b + [bh|bt].
  - x^T tiles produced via one SWDGE f32->bf16 cast into a DRAM scratch, then
    6 large xbar DMA transposes (one per 128-wide h chunk over all 4096 local
    tokens) split across the two HWDGE queues.
  - Token stats (mu/var) via PE: per 128-token tile, sum(x) and sum(x^2) as
    N=1 matmuls against a ones column (x^2 chunks squared on DVE), finalized
    batch-wide; interleaved into the tail of the LSTM recurrence.
  - LSTM: x-part pre-GEMMed for all 16 steps chunk-by-chunk as weights load;
    per-step h @ W_hh.T accumulates into strided slices of one PSUM tile.
"""

import functools
import os
import sys

import numpy as np

sys.path.insert(0, "/opt/trn_rl_repo")

import concourse.bass as bass
import concourse.tile as tile
from concourse import bacc, mybir
from concourse.bass import IndirectOffsetOnAxis
from concourse.tile_rust import add_dep_helper
from concourse.bass_utils import run_bass_kernel_spmd
from concourse.masks import make_identity

F32 = mybir.dt.float32
BF16 = mybir.dt.bfloat16
FP8 = mybir.dt.float8e4
I32 = mybir.dt.int32

B, S, H, R = 64, 512, 768, 51
T = 16                      # MAX_SPAN
NCORES = 8
BL = B // NCORES            # 8 local batches
KC = H // 128               # 6 contraction chunks
GM = 4 * H // 128           # 24 gate-dim chunks
NT = BL * S // 128          # 32 token tiles per core
BLS = BL * S                # 4096 local tokens
R2 = 2 * R                  # 102 combined head+tail outputs
EPS = 1e-12

FP8_WHH = True              # fp8 stationary W_hh (x8) with h/8 moving operand


def _kernel_body(tc):
    nc = tc.nc
    embed = nc.dram_tensor("embed", [BL, S, H], F32, kind="ExternalInput").ap()
    sub_head = nc.dram_tensor("sub_head", [BL], I32, kind="ExternalInput").ap()
    sub_tail = nc.dram_tensor("sub_tail", [BL], I32, kind="ExternalInput").ap()
    W_ih = nc.dram_tensor("W_ih", [4 * H, H], F32, kind="ExternalInput").ap()
    W_hh = nc.dram_tensor("W_hh", [4 * H, H], F32, kind="ExternalInput").ap()
    b_ih = nc.dram_tensor("b_ih", [4 * H], F32, kind="ExternalInput").ap()
    b_hh = nc.dram_tensor("b_hh", [4 * H], F32, kind="ExternalInput").ap()
    cln_w = nc.dram_tensor("cln_weight", [H], F32, kind="ExternalInput").ap()
    cln_b = nc.dram_tensor("cln_bias", [H], F32, kind="ExternalInput").ap()
    cln_Ww = nc.dram_tensor("cln_Ww", [H, H], F32, kind="ExternalInput").ap()
    cln_Wb = nc.dram_tensor("cln_Wb", [H, H], F32, kind="ExternalInput").ap()
    Wh = nc.dram_tensor("Wh", [R, H], F32, kind="ExternalInput").ap()
    bh = nc.dram_tensor("bh", [R], F32, kind="ExternalInput").ap()
    Wt = nc.dram_tensor("Wt", [R, H], F32, kind="ExternalInput").ap()
    bt = nc.dram_tensor("bt", [R], F32, kind="ExternalInput").ap()
    heads = nc.dram_tensor("heads", [BL, S, R], F32, kind="ExternalOutput").ap()
    tails = nc.dram_tensor("tails", [BL, S, R], F32, kind="ExternalOutput").ap()

    Sig = mybir.ActivationFunctionType.Sigmoid
    Tanh = mybir.ActivationFunctionType.Tanh
    Sqrt = mybir.ActivationFunctionType.Sqrt
    Copy = mybir.ActivationFunctionType.Copy
    NKB = KC * BL  # 48
    WH8DT = FP8 if FP8_WHH else BF16
    WH8SC = 8.0 if FP8_WHH else 1.0

    with (
        tc.tile_pool(name="persist", bufs=1) as pp,
        tc.tile_pool(name="wstage", bufs=2) as wstage,
        tc.tile_pool(name="wstage2", bufs=1) as wstage2,
        tc.tile_pool(name="xstage", bufs=3) as xstage,
        tc.tile_pool(name="sqstage", bufs=1) as sqstage,
        tc.tile_pool(name="dram", bufs=1, space="DRAM") as dram,
    ):
        # ---------------- constants / small prep ----------------
        ident = pp.tile([128, 128], F32, tag="ident")
        make_identity(nc, ident[:])
        ones_row = pp.tile([1, 128], BF16, tag="ones_row")
        nc.vector.memset(ones_row[:], 1.0)
        ones_col = pp.tile([128, 1], BF16, tag="ones_col")
        nc.vector.memset(ones_col[:], 1.0)

        head_sb = pp.tile([BL, 1], I32, tag="head_sb")
        tail_sb = pp.tile([BL, 1], I32, tag="tail_sb")
        nc.gpsimd.dma_start(head_sb[:], sub_head.rearrange("b -> b ()"))
        nc.gpsimd.dma_start(tail_sb[:], sub_tail.rearrange("b -> b ()"))

        # gather row offsets: off[b, t] = b*S + head_b + t  (embed flat [BL*S, H])
        iota_bs = pp.tile([BL, T], I32, tag="iota_bs")
        nc.gpsimd.iota(iota_bs[:], pattern=[[1, T]], base=0, channel_multiplier=S)
        off_bt = pp.tile([BL, T], I32, tag="off_bt")
        nc.vector.tensor_tensor(off_bt[:], iota_bs[:],
                                head_sb[:, 0:1].to_broadcast([BL, T]),
                                op=mybir.AluOpType.add)
        off_p = pp.tile([128, 1], I32, tag="off_p")
        nc.gpsimd.dma_start(off_p[:], off_bt[:])

        # span mask: mask[b, t] = (t <= tail_b - head_b)
        span = pp.tile([BL, 1], I32, tag="span")
        nc.vector.tensor_tensor(span[:], tail_sb[:], head_sb[:],
                                op=mybir.AluOpType.subtract)
        iota_t = pp.tile([BL, T], I32, tag="iota_t")
        nc.gpsimd.iota(iota_t[:], pattern=[[1, T]], base=0, channel_multiplier=0)
        mask_pad = pp.tile([32, 32], I32, tag="mask_pad")
        nc.vector.memset(mask_pad[:], 0)
        nc.vector.tensor_tensor(mask_pad[0:BL, 0:T], iota_t[:],
                                span[:, 0:1].to_broadcast([BL, T]),
                                op=mybir.AluOpType.is_le)
        maskTt = pp.tile([32, 32], I32, tag="maskTt")      # [t, b]
        nc.vector.transpose(maskTt[:], mask_pad[:])
        maskT = pp.tile([1, T * BL], I32, tag="maskT")     # col = t*8+b
        nc.gpsimd.dma_start(maskT[0:1, :], maskTt[0:T, 0:BL])
        mask_bc = pp.tile([128, T * BL], I32, tag="mask_bc")
        nc.gpsimd.partition_broadcast(mask_bc[:], maskT[0:1, :])
        # replicate over h-chunks: mask_bc6[p, t*48 + k*8 + b] = mask[b, t]
        mask_bc6 = pp.tile([128, T * KC * BL], mybir.dt.uint8, tag="mask_bc6")
        m6v = mask_bc6[:, :].rearrange("p (t k b) -> p k t b", k=KC, b=BL)
        mbv = mask_bc[:, :].rearrange("p (t b) -> p t b", b=BL)
        for k in range(KC):
            nc.vector.tensor_copy(m6v[:, k], mbv)

        # chunk-major small vectors: v_c[p, m] = v[m*128 + p]
        clnw_c = pp.tile([128, KC], F32, tag="clnw_c")
        clnb_c = pp.tile([128, KC], F32, tag="clnb_c")
        nc.gpsimd.dma_start(clnw_c[:], cln_w.rearrange("(k p) -> p k", p=128))
        nc.gpsimd.dma_start(clnb_c[:], cln_b.rearrange("(k p) -> p k", p=128))
        brow_c = pp.tile([128, GM], F32, tag="brow_c")
        nc.gpsimd.dma_start(brow_c[:], b_ih.rearrange("(m p) -> p m", p=128))
        nc.gpsimd.dma_start(brow_c[:], b_hh.rearrange("(m p) -> p m", p=128),
                            accum_op=mybir.AluOpType.add)
        bhbt_f = pp.tile([1, R2], F32, tag="bhbt_f")
        nc.gpsimd.dma_start(bhbt_f[:, 0:R], bh.rearrange("r -> () r"))
        nc.gpsimd.dma_start(bhbt_f[:, R:R2], bt.rearrange("r -> () r"))
        bhbt = pp.tile([1, R2], BF16, tag="bhbt")
        nc.vector.tensor_copy(bhbt[:], bhbt_f[:])

        # span gather for the LSTM input block (x[b, head_b+t, :])
        xsp_f = pp.tile([128, H], F32, tag="xsp_f")
        nc.gpsimd.indirect_dma_start(
            out=xsp_f[:], out_offset=None,
            in_=embed.rearrange("b s h -> (b s) h"),
            in_offset=IndirectOffsetOnAxis(ap=off_p[:, 0:1], axis=0))

        # DRAM scratch for the bf16 embed copy (cast issued after the W_hh
        # loads further down, so the weight DMAs get HBM first)
        ebf = dram.tile([BLS, H], BF16)

        # ---------------- persistent big tiles ----------------
        WihT = pp.tile([128, KC, 4 * H], BF16, tag="WihT")
        WhhT = pp.tile([128, KC, 4 * H], WH8DT, tag="WhhT")
        xT = pp.tile([128, KC, BLS], BF16, tag="xT")
        gates0 = pp.tile([128, GM * 128], BF16, tag="gates0")
        g0r = gates0[:, :].rearrange("p (m b t) -> p m b t", m=GM, b=BL)

        # LSTM state
        hT = pp.tile([128, NKB], F32, tag="hT")      # col = k*8 + b
        cT = pp.tile([128, NKB], F32, tag="cT")
        hbf = pp.tile([128, NKB], BF16, tag="hbf")
        gsig = pp.tile([128, 4 * NKB], F32, tag="gsig")  # i|f|o|tanh_g
        tmp1 = pp.tile([128, NKB], F32, tag="tmp1")
        tmp2 = pp.tile([128, NKB], F32, tag="tmp2")
        gates_f = pp.tile([128, GM * BL], F32, tag="gates_f")
        nc.vector.memset(hT[:], 0.0)
        nc.vector.memset(cT[:], 0.0)
        nc.vector.memset(hbf[:], 0.0)

        # token stats
        statsRaw = pp.tile([128, 2, NT], F32, tag="statsRaw")  # [sum|sumsq]
        mu_all = pp.tile([128, NT], F32, tag="mu_all")
        sa_all = pp.tile([128, NT], F32, tag="sa_all")     # std then (slot reuse)
        a_all = pp.tile([128, NT], F32, tag="a_all")
        tmp_nt = pp.tile([128, NT], F32, tag="tmp_nt")
        eps_t = pp.tile([128, 1], F32, tag="eps_t")
        nc.vector.memset(eps_t[:], EPS)

        with (
            tc.tile_pool(name="psum_w", bufs=2, space="PSUM") as pw,
            tc.tile_pool(name="psum_rec", bufs=2, space="PSUM") as psr,
            tc.tile_pool(name="psum_stats", bufs=2, space="PSUM") as pst,
        ):
            _cnt = [0]

            def psum_to_sbuf(dst, src):
                if _cnt[0] % 2 == 0:
                    nc.scalar.activation(dst, src, Copy, scale=1.0)
                else:
                    nc.vector.tensor_copy(dst, src)
                _cnt[0] += 1

            # xsp transpose on PE (needed by the pre-GEMM)
            psx = pw.tile([128, H], F32, tag="wtp")
            for k in range(KC):
                nc.tensor.transpose(psx[:, k * 128:(k + 1) * 128],
                                    xsp_f[:, k * 128:(k + 1) * 128], ident[:])
            xspT = pp.tile([128, KC, 128], BF16, tag="xspT")
            nc.scalar.activation(
                xspT[:, :, :], psx[:].rearrange("p (k c) -> p k c", k=KC), Copy)

            def load_wT_chunk(dst, src_ap, m0, weng, tag, scale=1.0):
                st = wstage.tile([128, 2, H], F32, tag=tag)
                st_dma = weng.dma_start(
                    st[:, :, :],
                    src_ap[m0 * 128:(m0 + 2) * 128, :]
                    .rearrange("(c p) h -> p c h", p=128))
                for c in range(2):
                    ps = pw.tile([128, H], F32, tag="wtp")
                    for k in range(KC):
                        nc.tensor.transpose(ps[:, k * 128:(k + 1) * 128],
                                            st[:, c, k * 128:(k + 1) * 128],
                                            ident[:])
                    m = m0 + c
                    if _cnt[0] % 2 == 0:
                        nc.scalar.activation(
                            dst[:, :, m * 128:(m + 1) * 128],
                            ps[:].rearrange("p (k c) -> p k c", k=KC), Copy,
                            scale=scale)
                    else:
                        if scale == 1.0:
                            nc.vector.tensor_copy(
                                dst[:, :, m * 128:(m + 1) * 128],
                                ps[:].rearrange("p (k c) -> p k c", k=KC))
                        else:
                            nc.vector.tensor_scalar_mul(
                                dst[:, :, m * 128:(m + 1) * 128],
                                ps[:].rearrange("p (k c) -> p k c", k=KC),
                                scale)
                    _cnt[0] += 1
                return st_dma

            # W_ih on the SP HWDGE queue, W_hh on the SWDGE queue (HWDGE
            # dma_start occupies the issuing sequencer for the whole
            # transfer, so keep ACT free for the PSUM->SBUF copies);
            # pre-GEMM for each W_ih pair right after it lands.
            for m0 in range(0, GM, 2):
                load_wT_chunk(WihT, W_ih, m0, nc.sync, "wstg_f")
                hh_dma = load_wT_chunk(WhhT, W_hh, m0, nc.gpsimd, "wstg_h",
                                       scale=WH8SC)
                for m in (m0, m0 + 1):
                    pg = pw.tile([128, H], F32, tag="wtp")
                    for k in range(KC):
                        nc.tensor.matmul(pg[:, 0:128],
                                         lhsT=WihT[:, k, m * 128:(m + 1) * 128],
                                         rhs=xspT[:, k, :],
                                         start=(k == 0), stop=(k == KC - 1))
                    # move to SBUF, adding (b_ih + b_hh)[m-chunk] per partition
                    nc.vector.tensor_scalar_add(
                        gates0[:, m * 128:(m + 1) * 128],
                        pg[:, 0:128], brow_c[:, m:m + 1])

            # bf16 cast of embed on SWDGE, pinned AFTER the last W_hh stage
            # transfer so the weight pipeline gets HBM bandwidth first.
            cast_ins = nc.gpsimd.dma_start(ebf[:, :],
                                           embed.rearrange("b s h -> (b s) h"))
            add_dep_helper(cast_ins.ins, hh_dma.ins, reason="cast after W_hh")

            # big xbar transposes DRAM(bf16) -> xT on the idle SP queue.
            # DRAM-space RAW on ebf may not be tracked by the tile dep hook,
            # so pin the cast -> transpose ordering explicitly.
            for k in range(KC):
                tr_ins = nc.sync.dma_start(xT[:, k, :],
                                           ebf[:, k * 128:(k + 1) * 128],
                                           transpose=True)
                add_dep_helper(tr_ins.ins, cast_ins.ins, reason="ebf RAW")

            # cln weights (reuse WihT's pool slot — free after pre-GEMM)
            WwbT = pp.tile([128, KC, 2 * H], BF16, tag="WihT")
            WwT = WwbT[:, :, 0:H]
            WbT = WwbT[:, :, H:2 * H]
            for m0 in range(0, KC, 2):
                load_wT_chunk(WwT, cln_Ww, m0, nc.gpsimd, "wstg_f")
            for m0 in range(0, KC, 2):
                load_wT_chunk(WbT, cln_Wb, m0, nc.gpsimd, "wstg_h")

            # Wh/Wt -> whwt[p, k, 0:102] = [Wh.T | Wt.T] bf16
            whwt = pp.tile([128, KC, R2], BF16, tag="whwt")
            wt_tp = pp.tile([128, KC, 128], BF16, tag="wt_tp")
            for src, half in ((Wh, 0), (Wt, 1)):
                wp = wstage2.tile([64, H], F32, tag="wstg_w")
                nc.vector.memset(wp[:], 0.0)
                nc.gpsimd.dma_start(wp[0:R, :], src[:, :])
                ps = pw.tile([128, H], F32, tag="wtp")
                for k in range(KC):
                    nc.tensor.transpose(ps[:, k * 64:(k + 1) * 64],
                                        wp[:, k * 128:(k + 1) * 128],
                                        ident[0:64, 0:64])
                nc.scalar.activation(
                    wt_tp[:, :, half * 64:half * 64 + 64],
                    ps[:, 0:KC * 64].rearrange("p (k c) -> p k c", k=KC), Copy)
            for k in range(KC):
                nc.vector.tensor_copy(
                    whwt[:, k, :].rearrange("p (w r) -> p w r", w=2),
                    wt_tp[:, k, :].rearrange("p (w r) -> p w r", w=2)[:, :, 0:R])

            # ---------------- LSTM recurrence (+ stats interleaved) -------
            for t in range(T):
                if t == 0:
                    # h0 = 0: gates come straight from the pre-GEMM
                    nc.scalar.activation(gsig[:, 0:2 * NKB],
                                         g0r[:, 0:12, :, 0], Sig)
                    nc.scalar.activation(gsig[:, 3 * NKB:4 * NKB],
                                         g0r[:, 12:18, :, 0], Tanh)
                    nc.scalar.activation(gsig[:, 2 * NKB:3 * NKB],
                                         g0r[:, 18:24, :, 0], Sig)
                else:
                    pr = psr.tile([128, GM * BL], F32, tag="pr")
                    for m in range(GM):
                        for k in range(KC):
                            nc.tensor.matmul(
                                pr[:, m * BL:(m + 1) * BL],
                                lhsT=WhhT[:, k, m * 128:(m + 1) * 128],
                                rhs=hbf[:, k * BL:(k + 1) * BL],
                                start=(k == 0), stop=(k == KC - 1))
                    # gates_t = h-part + x-part; split so i/f/g activations
                    # can overlap the o-chunk matmuls
                    nc.vector.tensor_add(
                        gates_f[:, 0:3 * NKB].rearrange("p (m b) -> p m b", m=18),
                        pr[:, 0:3 * NKB].rearrange("p (m b) -> p m b", m=18),
                        g0r[:, 0:18, :, t])
                    nc.scalar.activation(gsig[:, 0:2 * NKB],
                                         gates_f[:, 0:2 * NKB], Sig)
                    nc.scalar.activation(gsig[:, 3 * NKB:4 * NKB],
                                         gates_f[:, 2 * NKB:3 * NKB], Tanh)
                    nc.vector.tensor_add(
                        gates_f[:, 3 * NKB:4 * NKB]
                        .rearrange("p (m b) -> p m b", m=6),
                        pr[:, 3 * NKB:4 * NKB]
                        .rearrange("p (m b) -> p m b", m=6),
                        g0r[:, 18:24, :, t])
                    nc.scalar.activation(gsig[:, 2 * NKB:3 * NKB],
                                         gates_f[:, 3 * NKB:4 * NKB], Sig)
                msk = mask_bc6[:, t * NKB:(t + 1) * NKB]
                # c_new = sig_f*c + sig_i*tanh_g ; c = where(mask, c_new, c)
                nc.vector.tensor_mul(tmp1[:], gsig[:, NKB:2 * NKB], cT[:])
                nc.vector.tensor_mul(tmp2[:], gsig[:, 0:NKB],
                                     gsig[:, 3 * NKB:4 * NKB])
                nc.vector.tensor_add(tmp1[:], tmp1[:], tmp2[:])
                nc.vector.copy_predicated(cT[:], msk, tmp1[:])
                # h_new = sig_o * tanh(c_new) ; h = where(mask, h_new, h)
                nc.scalar.activation(tmp2[:], tmp1[:], Tanh)
                nc.vector.tensor_mul(tmp2[:], gsig[:, 2 * NKB:3 * NKB], tmp2[:])
                nc.vector.copy_predicated(hT[:], msk, tmp2[:])
                if FP8_WHH:
                    nc.vector.tensor_scalar_mul(hbf[:], hT[:], 1.0 / 8.0)
                else:
                    nc.vector.tensor_copy(hbf[:], hT[:])

                # interleave per-batch token stats into the recurrence tail
                # (last 4 steps, 2 batches each — xT lands ~100us in).
                # NOTE: each accumulation group gets its OWN psum tile (bank):
                # start=True clears has_written for the whole bank, so two
                # groups sharing a bank corrupt each other when the scheduler
                # interleaves them.
                if t >= T - BL // 2:
                    for g in (2 * (t - (T - BL // 2)),
                              2 * (t - (T - BL // 2)) + 1):
                        xsq = sqstage.tile([128, KC, S], BF16, tag="xsq")
                        for k in range(KC):
                            nc.vector.tensor_mul(xsq[:, k, :],
                                                 xT[:, k, g * S:(g + 1) * S],
                                                 xT[:, k, g * S:(g + 1) * S])
                        for c in range(4):
                            i = g * 4 + c
                            for x, src_t in (
                                    (0, xT[:, :, i * 128:(i + 1) * 128]),
                                    (1, xsq[:, :, c * 128:(c + 1) * 128])):
                                sp = pst.tile([128, 1], F32, tag="sp")
                                for k in range(KC):
                                    nc.tensor.matmul(
                                        sp[:, 0:1], lhsT=src_t[:, k, :],
                                        rhs=ones_col[:, 0:1],
                                        start=(k == 0), stop=(k == KC - 1))
                                psum_to_sbuf(statsRaw[:, x, i:i + 1],
                                             sp[:, 0:1])

        # ---------------- stats finalize ----------------
        with (
            tc.tile_pool(name="psum_small", bufs=2, space="PSUM") as pss,
            tc.tile_pool(name="psum_out", bufs=6, space="PSUM") as pso,
        ):
            nc.vector.tensor_scalar_mul(mu_all[:], statsRaw[:, 0, :], 1.0 / H)
            nc.vector.tensor_scalar_mul(sa_all[:], statsRaw[:, 1, :], 1.0 / H)
            nc.vector.tensor_mul(tmp_nt[:], mu_all[:], mu_all[:])
            nc.vector.tensor_tensor(sa_all[:], sa_all[:], tmp_nt[:],
                                    op=mybir.AluOpType.subtract)  # var
            nc.scalar.activation(sa_all[:], sa_all[:], Sqrt,
                                 bias=eps_t[:, 0:1])              # std
            nc.vector.reciprocal(a_all[:], sa_all[:])             # 1/std

            # ---------------- CLN projections ----------------
            wT = pp.tile([128, NKB], F32, tag="wT")
            bT = pp.tile([128, NKB], F32, tag="bT")
            for dst, wmat, aff in ((wT, WwT, clnw_c), (bT, WbT, clnb_c)):
                for ko in range(KC):
                    ps = pss.tile([128, BL], F32, tag="ps_small")
                    for ki in range(KC):
                        nc.tensor.matmul(ps[:],
                                         lhsT=wmat[:, ki, ko * 128:(ko + 1) * 128],
                                         rhs=hbf[:, ki * BL:(ki + 1) * BL],
                                         start=(ki == 0), stop=(ki == KC - 1))
                    nc.vector.tensor_scalar(dst[:, ko * BL:(ko + 1) * BL],
                                            ps[:], WH8SC, aff[:, ko:ko + 1],
                                            op0=mybir.AluOpType.mult,
                                            op1=mybir.AluOpType.add)
            bTb = pp.tile([128, NKB], BF16, tag="bTb")
            nc.vector.tensor_copy(bTb[:], bT[:])

            # ---------------- per-batch classifier params ----------------
            rhs_all = pp.tile([128, KC, BL, R2], BF16, tag="rhs_all")
            for k in range(KC):
                for b in range(BL):
                    nc.vector.tensor_scalar_mul(
                        rhs_all[:, k, b, :], whwt[:, k, :],
                        wT[:, k * BL + b:k * BL + b + 1])
            # vh_b = Wc_b @ 1 and c_b = [Wh|Wt] @ bvec_b + [bh|bt], replicated
            # across partitions for the token-major corrections
            vh_rep = pp.tile([128, BL, R2], F32, tag="vh_rep")
            c_rep = pp.tile([128, BL, R2], F32, tag="c_rep")
            for b in range(BL):
                ps = pss.tile([1, R2], F32, tag="ps_small")
                for k in range(KC):
                    nc.tensor.matmul(ps[:], lhsT=ones_col[:, 0:1],
                                     rhs=rhs_all[:, k, b, :],
                                     start=(k == 0), stop=(k == KC - 1))
                vrow = xstage.tile([128, R2], F32, tag="t1")
                nc.scalar.activation(vrow[0:1, :], ps[:], Copy)
                nc.gpsimd.partition_broadcast(vh_rep[:, b, :], vrow[0:1, :])
                ps2 = pss.tile([1, R2], F32, tag="ps_small")
                for k in range(KC):
                    nc.tensor.matmul(ps2[:],
                                     lhsT=bTb[:, k * BL + b:k * BL + b + 1],
                                     rhs=whwt[:, k, :], start=(k == 0),
                                     stop=False)
                nc.tensor.matmul(ps2[:], lhsT=ones_row[0:1, 0:1],
                                 rhs=bhbt[0:1, :], start=False, stop=True)
                crow = xstage.tile([128, R2], F32, tag="t1")
                nc.scalar.activation(crow[0:1, :], ps2[:], Copy)
                nc.gpsimd.partition_broadcast(c_rep[:, b, :], crow[0:1, :])

            # ---------------- classifier ----------------
            # logits = a * (G - mu x vh + std x c) ; a rides the sigmoid scale
            for i in range(NT):
                b, s0 = i // 4, (i % 4) * 128
                pt = pso.tile([128, R2], F32, tag="pt")
                for k in range(KC):
                    nc.tensor.matmul(pt[:],
                                     lhsT=xT[:, k, i * 128:(i + 1) * 128],
                                     rhs=rhs_all[:, k, b, :],
                                     start=(k == 0), stop=(k == KC - 1))
                t1 = xstage.tile([128, R2], F32, tag="t1")
                nc.vector.tensor_scalar_mul(t1[:], vh_rep[:, b, :],
                                            mu_all[:, i:i + 1])
                q = xstage.tile([128, R2], F32, tag="q")
                nc.vector.tensor_tensor(q[:], pt[:], t1[:],
                                        op=mybir.AluOpType.subtract)
                nc.vector.tensor_scalar_mul(t1[:], c_rep[:, b, :],
                                            sa_all[:, i:i + 1])
                nc.vector.tensor_add(q[:], q[:], t1[:])
                if i % 4 == 0:
                    out4 = xstage.tile([128, 4, R2], F32, tag="out4")
                nc.scalar.activation(out4[:, i % 4, :], q[:], Sig,
                                     scale=a_all[:, i:i + 1])
                if i % 4 == 3:
                    nc.sync.dma_start(
                        heads[b, :, :].rearrange("(c p) r -> p c r", p=128),
                        out4[:, :, 0:R])
                    nc.gpsimd.dma_start(
                        tails[b, :, :].rearrange("(c p) r -> p c r", p=128),
                        out4[:, :, R:R2])

            if os.environ.get("KDEBUG"):
                def dbg(name, ap, shape, dtype):
                    d = nc.dram_tensor(name, shape, dtype,
                                       kind="ExternalOutput").ap()
                    nc.sync.dma_start(d[:, :], ap)
                dbg("d_mu", mu_all[:, :], [128, NT], F32)
                dbg("d_std", sa_all[:, :], [128, NT], F32)
                dbg("d_aall", a_all[:, :], [128, NT], F32)
                dbg("d_hT", hT[:, :], [128, NKB], F32)
                dbg("d_wT", wT[:, :], [128, NKB], F32)
                dbg("d_bT", bT[:, :], [128, NKB], F32)
                dbg("d_xsp", xsp_f[:, :], [128, H], F32)
                dbg("d_xT", xT[:, 0, 0:256], [128, 256], BF16)
                dbg("d_xT5", xT[:, 5, 0:256], [128, 256], BF16)
                dbg("d_xTl", xT[:, 3, BLS - 256:BLS], [128, 256], BF16)
                dbg("d_sraw", statsRaw[:, :, :], [128, 2, NT], F32)


@functools.cache
def _build():
    nc = bacc.Bacc("TRN2", target_bir_lowering=False, debug=False,
                   enable_asserts=False, num_devices=NCORES)
    with tile.TileContext(nc) as tc:
        _kernel_body(tc)
    nc.compile()
    return nc


def kernel(**inputs):
    nc = _build()
    shared = {k: np.ascontiguousarray(np.asarray(inputs[k], dtype=np.float32))
              for k in ("W_ih", "W_hh", "b_ih", "b_hh", "cln_weight", "cln_bias",
                        "cln_Ww", "cln_Wb", "Wh", "bh", "Wt", "bt")}
    embed = np.ascontiguousarray(np.asarray(inputs["embed"], dtype=np.float32))
    sh = np.ascontiguousarray(np.asarray(inputs["sub_head"], dtype=np.int32))
    st = np.ascontiguousarray(np.asarray(inputs["sub_tail"], dtype=np.int32))
    in_maps = []
    for c in range(NCORES):
        sl = slice(c * BL, (c + 1) * BL)
        in_maps.append(dict(shared, embed=np.ascontiguousarray(embed[sl]),
                            sub_head=np.ascontiguousarray(sh[sl]),
                            sub_tail=np.ascontiguousarray(st[sl])))
    res = run_bass_kernel_spmd(nc, in_maps, list(range(NCORES)),
                               trace=bool(int(os.environ.get("KTRACE", "0"))))
    heads = np.concatenate([r["heads"] for r in res.results], axis=0)
    tails = np.concatenate([r["tails"] for r in res.results], axis=0)
    kernel.last_exec_time_ns = res.exec_time_ns
    return heads, tails


if __name__ == "__main__":
    np.random.seed(0)
    ins = {
        "embed": np.random.randn(B, S, H).astype(np.float32),
        "sub_head": np.random.randint(0, S - T, size=(B,)).astype(np.int32),
        "W_ih": (np.random.randn(4 * H, H) * 0.02).astype(np.float32),
        "W_hh": (np.random.randn(4 * H, H) * 0.02).astype(np.float32),
        "b_ih": np.zeros(4 * H, np.float32),
        "b_hh": np.zeros(4 * H, np.float32),
        "cln_weight": np.ones(H, np.float32),
        "cln_bias": np.zeros(H, np.float32),
        "cln_Ww": (np.random.randn(H, H) * 0.02).astype(np.float32),
        "cln_Wb": (np.random.randn(H, H) * 0.02).astype(np.float32),
        "Wh": (np.random.randn(R, H) * 0.02).astype(np.float32),
        "bh": np.zeros(R, np.float32),
        "Wt": (np.random.randn(R, H) * 0.02).astype(np.float32),
        "bt": np.zeros(R, np.float32),
    }
    ins["sub_tail"] = (ins["sub_head"]
                       + np.random.randint(0, T, size=(B,)).astype(np.int32))
    h, t = kernel(**ins)
    print("ok", h.shape, t.shape, h.dtype)


# revision 29
# speedup vs baseline: 1.0786x; 1.0786x over previous
"""CasRel-style kernel for Trainium2 (Bass/Tile), 8-core data-parallel.

Model (per batch b):
  hn_b      = masked LSTM over embed[b, head_b : head_b+16, :]  (16 steps, H=768)
  w_b       = hn_b @ cln_Ww.T + cln_weight ; bvec_b = hn_b @ cln_Wb.T + cln_bias
  ne[b,s]   = w_b * (x - mu)/std + bvec_b          (layernorm over H)
  heads     = sigmoid(ne @ Wh.T + bh) ; tails = sigmoid(ne @ Wt.T + bt)

Kernel strategy (per core, 8 local batches):
  - LN + classifier folded into one PSUM accumulation on raw x:
      logits = a_s * (Wc_b @ x_s - mu_s * vh_b + std_s * c_b),  a_s = 1/std_s
    where Wc_b = [Wh|Wt] * w_b (row-scaled in T layout), vh_b = Wc_b @ 1,
    c_b = [Wh|Wt] @ bvec_b + [bh|bt].
  - x^T tiles produced via one SWDGE f32->bf16 cast into a DRAM scratch, then
    6 large xbar DMA transposes (one per 128-wide h chunk over all 4096 local
    tokens) split across the two HWDGE queues.
  - Token stats (mu/var) via PE: per 128-token tile, sum(x) and sum(x^2) as
    N=1 matmuls against a ones column (x^2 chunks squared on DVE), finalized
    batch-wide; interleaved into the tail of the LSTM recurrence.
  - LSTM: x-part pre-GEMMed for all 16 steps chunk-by-chunk as weights load;
    per-step h @ W_hh.T accumulates into strided slices of one PSUM tile.
"""

import functools
import os
import sys

import numpy as np

sys.path.insert(0, "/opt/trn_rl_repo")

import concourse.bass as bass
import concourse.tile as tile
from concourse import bacc, mybir
from concourse.bass import IndirectOffsetOnAxis
from concourse.tile_rust import add_dep_helper
from concourse.bass_utils import run_bass_kernel_spmd
from concourse.masks import make_identity

F32 = mybir.dt.float32
BF16 = mybir.dt.bfloat16
FP8 = mybir.dt.float8e4
I32 = mybir.dt.int32

B, S, H, R = 64, 512, 768, 51
T = 16                      # MAX_SPAN
NCORES = 8
BL = B // NCORES            # 8 local batches
KC = H // 128               # 6 contraction chunks
GM = 4 * H // 128           # 24 gate-dim chunks
NT = BL * S // 128          # 32 token tiles per core
BLS = BL * S                # 4096 local tokens
R2 = 2 * R                  # 102 combined head+tail outputs
EPS = 1e-12

FP8_WHH = True              # fp8 stationary W_hh (x8) with h/8 moving operand


def _kernel_body(tc):
    nc = tc.nc
    embed = nc.dram_tensor("embed", [BL, S, H], F32, kind="ExternalInput").ap()
    sub_head = nc.dram_tensor("sub_head", [BL], I32, kind="ExternalInput").ap()
    sub_tail = nc.dram_tensor("sub_tail", [BL], I32, kind="ExternalInput").ap()
    W_ih = nc.dram_tensor("W_ih", [4 * H, H], F32, kind="ExternalInput").ap()
    W_hh = nc.dram_tensor("W_hh", [4 * H, H], F32, kind="ExternalInput").ap()
    b_ih = nc.dram_tensor("b_ih", [4 * H], F32, kind="ExternalInput").ap()
    b_hh = nc.dram_tensor("b_hh", [4 * H], F32, kind="ExternalInput").ap()
    cln_w = nc.dram_tensor("cln_weight", [H], F32, kind="ExternalInput").ap()
    cln_b = nc.dram_tensor("cln_bias", [H], F32, kind="ExternalInput").ap()
    cln_Ww = nc.dram_tensor("cln_Ww", [H, H], F32, kind="ExternalInput").ap()
    cln_Wb = nc.dram_tensor("cln_Wb", [H, H], F32, kind="ExternalInput").ap()
    Wh = nc.dram_tensor("Wh", [R, H], F32, kind="ExternalInput").ap()
    bh = nc.dram_tensor("bh", [R], F32, kind="ExternalInput").ap()
    Wt = nc.dram_tensor("Wt", [R, H], F32, kind="ExternalInput").ap()
    bt = nc.dram_tensor("bt", [R], F32, kind="ExternalInput").ap()
    heads = nc.dram_tensor("heads", [BL, S, R], F32, kind="ExternalOutput").ap()
    tails = nc.dram_tensor("tails", [BL, S, R], F32, kind="ExternalOutput").ap()

    Sig = mybir.ActivationFunctionType.Sigmoid
    Tanh = mybir.ActivationFunctionType.Tanh
    Sqrt = mybir.ActivationFunctionType.Sqrt
    Copy = mybir.ActivationFunctionType.Copy
    NKB = KC * BL  # 48
    WH8DT = FP8 if FP8_WHH else BF16
    WH8SC = 8.0 if FP8_WHH else 1.0

    with (
        tc.tile_pool(name="persist", bufs=1) as pp,
        tc.tile_pool(name="wstage", bufs=2) as wstage,
        tc.tile_pool(name="wstage2", bufs=1) as wstage2,
        tc.tile_pool(name="xstage", bufs=2) as xstage,
        tc.tile_pool(name="sqstage", bufs=1) as sqstage,
        tc.tile_pool(name="dram", bufs=1, space="DRAM") as dram,
    ):
        # ---------------- constants / small prep ----------------
        ident = pp.tile([128, 128], F32, tag="ident")
        make_identity(nc, ident[:])
        ones_row = pp.tile([1, 128], BF16, tag="ones_row")
        nc.vector.memset(ones_row[:], 1.0)
        ones_col = pp.tile([128, 1], BF16, tag="ones_col")
        nc.vector.memset(ones_col[:], 1.0)

        head_sb = pp.tile([BL, 1], I32, tag="head_sb")
        tail_sb = pp.tile([BL, 1], I32, tag="tail_sb")
        nc.gpsimd.dma_start(head_sb[:], sub_head.rearrange("b -> b ()"))
        nc.gpsimd.dma_start(tail_sb[:], sub_tail.rearrange("b -> b ()"))

        # gather row offsets: off[b, t] = b*S + head_b + t  (embed flat [BL*S, H])
        iota_bs = pp.tile([BL, T], I32, tag="iota_bs")
        nc.gpsimd.iota(iota_bs[:], pattern=[[1, T]], base=0, channel_multiplier=S)
        off_bt = pp.tile([BL, T], I32, tag="off_bt")
        nc.vector.tensor_tensor(off_bt[:], iota_bs[:],
                                head_sb[:, 0:1].to_broadcast([BL, T]),
                                op=mybir.AluOpType.add)
        off_p = pp.tile([128, 1], I32, tag="off_p")
        nc.gpsimd.dma_start(off_p[:], off_bt[:])

        # span mask: mask[b, t] = (t <= tail_b - head_b)
        span = pp.tile([BL, 1], I32, tag="span")
        nc.vector.tensor_tensor(span[:], tail_sb[:], head_sb[:],
                                op=mybir.AluOpType.subtract)
        iota_t = pp.tile([BL, T], I32, tag="iota_t")
        nc.gpsimd.iota(iota_t[:], pattern=[[1, T]], base=0, channel_multiplier=0)
        mask_pad = pp.tile([32, 32], I32, tag="mask_pad")
        nc.vector.memset(mask_pad[:], 0)
        nc.vector.tensor_tensor(mask_pad[0:BL, 0:T], iota_t[:],
                                span[:, 0:1].to_broadcast([BL, T]),
                                op=mybir.AluOpType.is_le)
        maskTt = pp.tile([32, 32], I32, tag="maskTt")      # [t, b]
        nc.vector.transpose(maskTt[:], mask_pad[:])
        maskT = pp.tile([1, T * BL], I32, tag="maskT")     # col = t*8+b
        nc.gpsimd.dma_start(maskT[0:1, :], maskTt[0:T, 0:BL])
        mask_bc = pp.tile([128, T * BL], I32, tag="mask_bc")
        nc.gpsimd.partition_broadcast(mask_bc[:], maskT[0:1, :])
        # replicate over h-chunks: mask_bc6[p, t*48 + k*8 + b] = mask[b, t]
        mask_bc6 = pp.tile([128, T * KC * BL], mybir.dt.uint8, tag="mask_bc6")
        m6v = mask_bc6[:, :].rearrange("p (t k b) -> p k t b", k=KC, b=BL)
        mbv = mask_bc[:, :].rearrange("p (t b) -> p t b", b=BL)
        for k in range(KC):
            nc.vector.tensor_copy(m6v[:, k], mbv)

        # chunk-major small vectors: v_c[p, m] = v[m*128 + p]
        clnw_c = pp.tile([128, KC], F32, tag="clnw_c")
        clnb_c = pp.tile([128, KC], F32, tag="clnb_c")
        nc.gpsimd.dma_start(clnw_c[:], cln_w.rearrange("(k p) -> p k", p=128))
        nc.gpsimd.dma_start(clnb_c[:], cln_b.rearrange("(k p) -> p k", p=128))
        brow_c = pp.tile([128, GM], F32, tag="brow_c")
        nc.gpsimd.dma_start(brow_c[:], b_ih.rearrange("(m p) -> p m", p=128))
        nc.gpsimd.dma_start(brow_c[:], b_hh.rearrange("(m p) -> p m", p=128),
                            accum_op=mybir.AluOpType.add)
        bhbt_f = pp.tile([1, R2], F32, tag="bhbt_f")
        nc.gpsimd.dma_start(bhbt_f[:, 0:R], bh.rearrange("r -> () r"))
        nc.gpsimd.dma_start(bhbt_f[:, R:R2], bt.rearrange("r -> () r"))
        bhbt = pp.tile([1, R2], BF16, tag="bhbt")
        nc.vector.tensor_copy(bhbt[:], bhbt_f[:])

        # span gather for the LSTM input block (x[b, head_b+t, :])
        xsp_f = pp.tile([128, H], F32, tag="xsp_f")
        nc.gpsimd.indirect_dma_start(
            out=xsp_f[:], out_offset=None,
            in_=embed.rearrange("b s h -> (b s) h"),
            in_offset=IndirectOffsetOnAxis(ap=off_p[:, 0:1], axis=0))

        # DRAM scratch for the bf16 embed copy (cast issued after the W_hh
        # loads further down, so the weight DMAs get HBM first)
        ebf = dram.tile([BLS, H], BF16)

        # ---------------- persistent big tiles ----------------
        WihT = pp.tile([128, KC, 4 * H], BF16, tag="WihT")
        WhhT = pp.tile([128, KC, 4 * H], WH8DT, tag="WhhT")
        xT = pp.tile([128, KC, BLS], BF16, tag="xT")
        gates0 = pp.tile([128, GM * 128], BF16, tag="gates0")
        g0r = gates0[:, :].rearrange("p (m b t) -> p m b t", m=GM, b=BL)

        # LSTM state
        hT = pp.tile([128, NKB], F32, tag="hT")      # col = k*8 + b
        cT = pp.tile([128, NKB], F32, tag="cT")
        hbf = pp.tile([128, NKB], BF16, tag="hbf")
        gsig = pp.tile([128, 4 * NKB], F32, tag="gsig")  # i|f|o|tanh_g
        tmp1 = pp.tile([128, NKB], F32, tag="tmp1")
        tmp2 = pp.tile([128, NKB], F32, tag="tmp2")
        gates_f = pp.tile([128, GM * BL], F32, tag="gates_f")
        nc.vector.memset(hT[:], 0.0)
        nc.vector.memset(cT[:], 0.0)
        nc.vector.memset(hbf[:], 0.0)

        # token stats
        statsRaw = pp.tile([128, 2, NT], F32, tag="statsRaw")  # [sum|sumsq]
        mu_all = pp.tile([128, NT], F32, tag="mu_all")
        sa_all = pp.tile([128, NT], F32, tag="sa_all")     # std then (slot reuse)
        a_all = pp.tile([128, NT], F32, tag="a_all")
        tmp_nt = pp.tile([128, NT], F32, tag="tmp_nt")
        eps_t = pp.tile([128, 1], F32, tag="eps_t")
        nc.vector.memset(eps_t[:], EPS)

        with (
            tc.tile_pool(name="psum_w", bufs=2, space="PSUM") as pw,
            tc.tile_pool(name="psum_rec", bufs=2, space="PSUM") as psr,
            tc.tile_pool(name="psum_stats", bufs=2, space="PSUM") as pst,
        ):
            _cnt = [0]

            def psum_to_sbuf(dst, src):
                if _cnt[0] % 2 == 0:
                    nc.scalar.activation(dst, src, Copy, scale=1.0)
                else:
                    nc.vector.tensor_copy(dst, src)
                _cnt[0] += 1

            # xsp transpose on PE (needed by the pre-GEMM)
            psx = pw.tile([128, H], F32, tag="wtp")
            for k in range(KC):
                nc.tensor.transpose(psx[:, k * 128:(k + 1) * 128],
                                    xsp_f[:, k * 128:(k + 1) * 128], ident[:])
            xspT = pp.tile([128, KC, 128], BF16, tag="xspT")
            nc.scalar.activation(
                xspT[:, :, :], psx[:].rearrange("p (k c) -> p k c", k=KC), Copy)

            def load_wT_chunk(dst, src_ap, m0, weng, tag, scale=1.0):
                st = wstage.tile([128, 2, H], F32, tag=tag)
                st_dma = weng.dma_start(
                    st[:, :, :],
                    src_ap[m0 * 128:(m0 + 2) * 128, :]
                    .rearrange("(c p) h -> p c h", p=128))
                for c in range(2):
                    ps = pw.tile([128, H], F32, tag="wtp")
                    for k in range(KC):
                        nc.tensor.transpose(ps[:, k * 128:(k + 1) * 128],
                                            st[:, c, k * 128:(k + 1) * 128],
                                            ident[:])
                    m = m0 + c
                    if _cnt[0] % 2 == 0:
                        nc.scalar.activation(
                            dst[:, :, m * 128:(m + 1) * 128],
                            ps[:].rearrange("p (k c) -> p k c", k=KC), Copy,
                            scale=scale)
                    else:
                        if scale == 1.0:
                            nc.vector.tensor_copy(
                                dst[:, :, m * 128:(m + 1) * 128],
                                ps[:].rearrange("p (k c) -> p k c", k=KC))
                        else:
                            nc.vector.tensor_scalar_mul(
                                dst[:, :, m * 128:(m + 1) * 128],
                                ps[:].rearrange("p (k c) -> p k c", k=KC),
                                scale)
                    _cnt[0] += 1
                return st_dma

            # W_ih on the SP HWDGE queue, W_hh on the SWDGE queue (HWDGE
            # dma_start occupies the issuing sequencer for the whole
            # transfer, so keep ACT free for the PSUM->SBUF copies);
            # pre-GEMM for each W_ih pair right after it lands.
            for m0 in range(0, GM, 2):
                load_wT_chunk(WihT, W_ih, m0, nc.sync, "wstg_f")
                load_wT_chunk(WhhT, W_hh, m0, nc.gpsimd, "wstg_h",
                              scale=WH8SC)
                for m in (m0, m0 + 1):
                    pg = pw.tile([128, H], F32, tag="wtp")
                    for k in range(KC):
                        nc.tensor.matmul(pg[:, 0:128],
                                         lhsT=WihT[:, k, m * 128:(m + 1) * 128],
                                         rhs=xspT[:, k, :],
                                         start=(k == 0), stop=(k == KC - 1))
                    # move to SBUF, adding (b_ih + b_hh)[m-chunk] per partition
                    nc.vector.tensor_scalar_add(
                        gates0[:, m * 128:(m + 1) * 128],
                        pg[:, 0:128], brow_c[:, m:m + 1])

            # bf16 cast of embed on SWDGE, queued behind the W_hh loads
            cast_ins = nc.gpsimd.dma_start(ebf[:, :],
                                           embed.rearrange("b s h -> (b s) h"))

            # big xbar transposes DRAM(bf16) -> xT split over both HWDGE
            # queues. DRAM-space RAW on ebf may not be tracked by the tile
            # dep hook, so pin the cast -> transpose ordering explicitly.
            for k in range(KC):
                xeng = nc.sync if k % 2 == 0 else nc.scalar
                tr_ins = xeng.dma_start(xT[:, k, :],
                                        ebf[:, k * 128:(k + 1) * 128],
                                        transpose=True)
                add_dep_helper(tr_ins.ins, cast_ins.ins, reason="ebf RAW")

            # cln weights (reuse WihT's pool slot — free after pre-GEMM)
            WwbT = pp.tile([128, KC, 2 * H], BF16, tag="WihT")
            WwT = WwbT[:, :, 0:H]
            WbT = WwbT[:, :, H:2 * H]
            for m0 in range(0, KC, 2):
                load_wT_chunk(WwT, cln_Ww, m0, nc.gpsimd, "wstg_f")
            for m0 in range(0, KC, 2):
                load_wT_chunk(WbT, cln_Wb, m0, nc.gpsimd, "wstg_h")

            # Wh/Wt -> whwt[p, k, 0:102] = [Wh.T | Wt.T] bf16
            whwt = pp.tile([128, KC, R2], BF16, tag="whwt")
            wt_tp = pp.tile([128, KC, 128], BF16, tag="wt_tp")
            for src, half in ((Wh, 0), (Wt, 1)):
                wp = wstage2.tile([64, H], F32, tag="wstg_w")
                nc.vector.memset(wp[:], 0.0)
                nc.gpsimd.dma_start(wp[0:R, :], src[:, :])
                ps = pw.tile([128, H], F32, tag="wtp")
                for k in range(KC):
                    nc.tensor.transpose(ps[:, k * 64:(k + 1) * 64],
                                        wp[:, k * 128:(k + 1) * 128],
                                        ident[0:64, 0:64])
                nc.scalar.activation(
                    wt_tp[:, :, half * 64:half * 64 + 64],
                    ps[:, 0:KC * 64].rearrange("p (k c) -> p k c", k=KC), Copy)
            for k in range(KC):
                nc.vector.tensor_copy(
                    whwt[:, k, :].rearrange("p (w r) -> p w r", w=2),
                    wt_tp[:, k, :].rearrange("p (w r) -> p w r", w=2)[:, :, 0:R])

            # ---------------- LSTM recurrence (+ stats interleaved) -------
            for t in range(T):
                if t == 0:
                    # h0 = 0: gates come straight from the pre-GEMM
                    nc.scalar.activation(gsig[:, 0:2 * NKB],
                                         g0r[:, 0:12, :, 0], Sig)
                    nc.scalar.activation(gsig[:, 3 * NKB:4 * NKB],
                                         g0r[:, 12:18, :, 0], Tanh)
                    nc.scalar.activation(gsig[:, 2 * NKB:3 * NKB],
                                         g0r[:, 18:24, :, 0], Sig)
                else:
                    pr = psr.tile([128, GM * BL], F32, tag="pr")
                    for m in range(GM):
                        for k in range(KC):
                            nc.tensor.matmul(
                                pr[:, m * BL:(m + 1) * BL],
                                lhsT=WhhT[:, k, m * 128:(m + 1) * 128],
                                rhs=hbf[:, k * BL:(k + 1) * BL],
                                start=(k == 0), stop=(k == KC - 1))
                    # gates_t = h-part + x-part; split so i/f/g activations
                    # can overlap the o-chunk matmuls
                    nc.vector.tensor_add(
                        gates_f[:, 0:3 * NKB].rearrange("p (m b) -> p m b", m=18),
                        pr[:, 0:3 * NKB].rearrange("p (m b) -> p m b", m=18),
                        g0r[:, 0:18, :, t])
                    nc.scalar.activation(gsig[:, 0:2 * NKB],
                                         gates_f[:, 0:2 * NKB], Sig)
                    nc.scalar.activation(gsig[:, 3 * NKB:4 * NKB],
                                         gates_f[:, 2 * NKB:3 * NKB], Tanh)
                    nc.vector.tensor_add(
                        gates_f[:, 3 * NKB:4 * NKB]
                        .rearrange("p (m b) -> p m b", m=6),
                        pr[:, 3 * NKB:4 * NKB]
                        .rearrange("p (m b) -> p m b", m=6),
                        g0r[:, 18:24, :, t])
                    nc.scalar.activation(gsig[:, 2 * NKB:3 * NKB],
                                         gates_f[:, 3 * NKB:4 * NKB], Sig)
                msk = mask_bc6[:, t * NKB:(t + 1) * NKB]
                # c_new = sig_f*c + sig_i*tanh_g ; c = where(mask, c_new, c)
                nc.vector.tensor_mul(tmp1[:], gsig[:, NKB:2 * NKB], cT[:])
                nc.vector.tensor_mul(tmp2[:], gsig[:, 0:NKB],
                                     gsig[:, 3 * NKB:4 * NKB])
                nc.vector.tensor_add(tmp1[:], tmp1[:], tmp2[:])
                nc.vector.copy_predicated(cT[:], msk, tmp1[:])
                # h_new = sig_o * tanh(c_new) ; h = where(mask, h_new, h)
                nc.scalar.activation(tmp2[:], tmp1[:], Tanh)
                nc.vector.tensor_mul(tmp2[:], gsig[:, 2 * NKB:3 * NKB], tmp2[:])
                nc.vector.copy_predicated(hT[:], msk, tmp2[:])
                if FP8_WHH:
                    nc.vector.tensor_scalar_mul(hbf[:], hT[:], 1.0 / 8.0)
                else:
                    nc.vector.tensor_copy(hbf[:], hT[:])

                # interleave per-batch token stats into the recurrence tail
                # (last 4 steps, 2 batches each — xT lands ~100us in).
                # NOTE: each accumulation group gets its OWN psum tile (bank):
                # start=True clears has_written for the whole bank, so two
                # groups sharing a bank corrupt each other when the scheduler
                # interleaves them.
                if t >= T - BL:
                    for g in (t - (T - BL),):
                        xsq = sqstage.tile([128, KC, S], BF16, tag="xsq")
                        for k in range(KC):
                            nc.vector.tensor_mul(xsq[:, k, :],
                                                 xT[:, k, g * S:(g + 1) * S],
                                                 xT[:, k, g * S:(g + 1) * S])
                        for c in range(4):
                            i = g * 4 + c
                            for x, src_t in (
                                    (0, xT[:, :, i * 128:(i + 1) * 128]),
                                    (1, xsq[:, :, c * 128:(c + 1) * 128])):
                                sp = pst.tile([128, 1], F32, tag="sp")
                                for k in range(KC):
                                    nc.tensor.matmul(
                                        sp[:, 0:1], lhsT=src_t[:, k, :],
                                        rhs=ones_col[:, 0:1],
                                        start=(k == 0), stop=(k == KC - 1))
                                psum_to_sbuf(statsRaw[:, x, i:i + 1],
                                             sp[:, 0:1])

        # ---------------- stats finalize ----------------
        with (
            tc.tile_pool(name="psum_small", bufs=2, space="PSUM") as pss,
            tc.tile_pool(name="psum_out", bufs=4, space="PSUM") as pso,
        ):
            nc.vector.tensor_scalar_mul(mu_all[:], statsRaw[:, 0, :], 1.0 / H)
            nc.vector.tensor_scalar_mul(sa_all[:], statsRaw[:, 1, :], 1.0 / H)
            nc.vector.tensor_mul(tmp_nt[:], mu_all[:], mu_all[:])
            nc.vector.tensor_tensor(sa_all[:], sa_all[:], tmp_nt[:],
                                    op=mybir.AluOpType.subtract)  # var
            nc.scalar.activation(sa_all[:], sa_all[:], Sqrt,
                                 bias=eps_t[:, 0:1])              # std
            nc.vector.reciprocal(a_all[:], sa_all[:])             # 1/std

            # ---------------- CLN projections ----------------
            wT = pp.tile([128, NKB], F32, tag="wT")
            bT = pp.tile([128, NKB], F32, tag="bT")
            for dst, wmat, aff in ((wT, WwT, clnw_c), (bT, WbT, clnb_c)):
                for ko in range(KC):
                    ps = pss.tile([128, BL], F32, tag="ps_small")
                    for ki in range(KC):
                        nc.tensor.matmul(ps[:],
                                         lhsT=wmat[:, ki, ko * 128:(ko + 1) * 128],
                                         rhs=hbf[:, ki * BL:(ki + 1) * BL],
                                         start=(ki == 0), stop=(ki == KC - 1))
                    nc.vector.tensor_scalar(dst[:, ko * BL:(ko + 1) * BL],
                                            ps[:], WH8SC, aff[:, ko:ko + 1],
                                            op0=mybir.AluOpType.mult,
                                            op1=mybir.AluOpType.add)
            bTb = pp.tile([128, NKB], BF16, tag="bTb")
            nc.vector.tensor_copy(bTb[:], bT[:])

            # ---------------- per-batch classifier params ----------------
            rhs_all = pp.tile([128, KC, BL, R2], BF16, tag="rhs_all")
            for k in range(KC):
                for b in range(BL):
                    nc.vector.tensor_scalar_mul(
                        rhs_all[:, k, b, :], whwt[:, k, :],
                        wT[:, k * BL + b:k * BL + b + 1])
            # vh_b = Wc_b @ 1 and c_b = [Wh|Wt] @ bvec_b + [bh|bt], replicated
            # across partitions for the token-major corrections
            vh_rep = pp.tile([128, BL, R2], F32, tag="vh_rep")
            c_rep = pp.tile([128, BL, R2], F32, tag="c_rep")
            for b in range(BL):
                ps = pss.tile([1, R2], F32, tag="ps_small")
                for k in range(KC):
                    nc.tensor.matmul(ps[:], lhsT=ones_col[:, 0:1],
                                     rhs=rhs_all[:, k, b, :],
                                     start=(k == 0), stop=(k == KC - 1))
                vrow = xstage.tile([128, R2], F32, tag="t1")
                nc.scalar.activation(vrow[0:1, :], ps[:], Copy)
                nc.gpsimd.partition_broadcast(vh_rep[:, b, :], vrow[0:1, :])
                ps2 = pss.tile([1, R2], F32, tag="ps_small")
                for k in range(KC):
                    nc.tensor.matmul(ps2[:],
                                     lhsT=bTb[:, k * BL + b:k * BL + b + 1],
                                     rhs=whwt[:, k, :], start=(k == 0),
                                     stop=False)
                nc.tensor.matmul(ps2[:], lhsT=ones_row[0:1, 0:1],
                                 rhs=bhbt[0:1, :], start=False, stop=True)
                crow = xstage.tile([128, R2], F32, tag="t1")
                nc.scalar.activation(crow[0:1, :], ps2[:], Copy)
                nc.gpsimd.partition_broadcast(c_rep[:, b, :], crow[0:1, :])

            # ---------------- classifier ----------------
            # logits = a * (G - mu x vh + std x c) ; a rides the sigmoid scale
            for i in range(NT):
                b, s0 = i // 4, (i % 4) * 128
                pt = pso.tile([128, R2], F32, tag="pt")
                for k in range(KC):
                    nc.tensor.matmul(pt[:],
                                     lhsT=xT[:, k, i * 128:(i + 1) * 128],
                                     rhs=rhs_all[:, k, b, :],
                                     start=(k == 0), stop=(k == KC - 1))
                t1 = xstage.tile([128, R2], F32, tag="t1")
                nc.vector.tensor_scalar_mul(t1[:], vh_rep[:, b, :],
                                            mu_all[:, i:i + 1])
                q = xstage.tile([128, R2], F32, tag="q")
                nc.vector.tensor_tensor(q[:], pt[:], t1[:],
                                        op=mybir.AluOpType.subtract)
                nc.vector.tensor_scalar_mul(t1[:], c_rep[:, b, :],
                                            sa_all[:, i:i + 1])
                nc.vector.tensor_add(q[:], q[:], t1[:])
                if i % 4 == 0:
                    out4 = xstage.tile([128, 4, R2], F32, tag="out4")
                nc.scalar.activation(out4[:, i % 4, :], q[:], Sig,
                                     scale=a_all[:, i:i + 1])
                if i % 4 == 3:
                    nc.sync.dma_start(
                        heads[b, :, :].rearrange("(c p) r -> p c r", p=128),
                        out4[:, :, 0:R])
                    nc.gpsimd.dma_start(
                        tails[b, :, :].rearrange("(c p) r -> p c r", p=128),
                        out4[:, :, R:R2])

            if os.environ.get("KDEBUG"):
                def dbg(name, ap, shape, dtype):
                    d = nc.dram_tensor(name, shape, dtype,
                                       kind="ExternalOutput").ap()
                    nc.sync.dma_start(d[:, :], ap)
                dbg("d_mu", mu_all[:, :], [128, NT], F32)
                dbg("d_std", sa_all[:, :], [128, NT], F32)
                dbg("d_aall", a_all[:, :], [128, NT], F32)
                dbg("d_hT", hT[:, :], [128, NKB], F32)
                dbg("d_wT", wT[:, :], [128, NKB], F32)
                dbg("d_bT", bT[:, :], [128, NKB], F32)
                dbg("d_xsp", xsp_f[:, :], [128, H], F32)
                dbg("d_xT", xT[:, 0, 0:256], [128, 256], BF16)
                dbg("d_xT5", xT[:, 5, 0:256], [128, 256], BF16)
                dbg("d_xTl", xT[:, 3, BLS - 256:BLS], [128, 256], BF16)
                dbg("d_sraw", statsRaw[:, :, :], [128, 2, NT], F32)


@functools.cache
def _build():
    nc = bacc.Bacc("TRN2", target_bir_lowering=False, debug=False,
                   enable_asserts=False, num_devices=NCORES)
    with tile.TileContext(nc) as tc:
        _kernel_body(tc)
    nc.compile()
    return nc


def kernel(**inputs):
    nc = _build()
    shared = {k: np.ascontiguousarray(np.asarray(inputs[k], dtype=np.float32))
              for k in ("W_ih", "W_hh", "b_ih", "b_hh", "cln_weight", "cln_bias",
                        "cln_Ww", "cln_Wb", "Wh", "bh", "Wt", "bt")}
    embed = np.ascontiguousarray(np.asarray(inputs["embed"], dtype=np.float32))
    sh = np.ascontiguousarray(np.asarray(inputs["sub_head"], dtype=np.int32))
    st = np.ascontiguousarray(np.asarray(inputs["sub_tail"], dtype=np.int32))
    in_maps = []
    for c in range(NCORES):
        sl = slice(c * BL, (c + 1) * BL)
        in_maps.append(dict(shared, embed=np.ascontiguousarray(embed[sl]),
                            sub_head=np.ascontiguousarray(sh[sl]),
                            sub_tail=np.ascontiguousarray(st[sl])))
    res = run_bass_kernel_spmd(nc, in_maps, list(range(NCORES)),
                               trace=bool(int(os.environ.get("KTRACE", "0"))))
    heads = np.concatenate([r["heads"] for r in res.results], axis=0)
    tails = np.concatenate([r["tails"] for r in res.results], axis=0)
    kernel.last_exec_time_ns = res.exec_time_ns
    return heads, tails


if __name__ == "__main__":
    np.random.seed(0)
    ins = {
        "embed": np.random.randn(B, S, H).astype(np.float32),
        "sub_head": np.random.randint(0, S - T, size=(B,)).astype(np.int32),
        "W_ih": (np.random.randn(4 * H, H) * 0.02).astype(np.float32),
        "W_hh": (np.random.randn(4 * H, H) * 0.02).astype(np.float32),
        "b_ih": np.zeros(4 * H, np.float32),
        "b_hh": np.zeros(4 * H, np.float32),
        "cln_weight": np.ones(H, np.float32),
        "cln_bias": np.zeros(H, np.float32),
        "cln_Ww": (np.random.randn(H, H) * 0.02).astype(np.float32),
        "cln_Wb": (np.random.randn(H, H) * 0.02).astype(np.float32),
        "Wh": (np.random.randn(R, H) * 0.02).astype(np.float32),
        "bh": np.zeros(R, np.float32),
        "Wt": (np.random.randn(R, H) * 0.02).astype(np.float32),
        "bt": np.zeros(R, np.float32),
    }
    ins["sub_tail"] = (ins["sub_head"]
                       + np.random.randint(0, T, size=(B,)).astype(np.int32))
    h, t = kernel(**ins)
    print("ok", h.shape, t.shape, h.dtype)
